# revision 2
# baseline (speedup 1.0000x reference)
"""TRN2 Bass kernel v2: bf16 symmetric-half weighted Gram with f32 diag.

cov = (x * cov_kernel) @ x^T with diag := softplus(x @ var_kernel + var_bias) + 1e-8

Same 16-stripe half-matrix scheme as v1 (core i runs adjacent stripes 2i,
2i+1; host mirrors the other half by transposition), but the whole device
pipeline runs in bf16, which the 2e-2 rel-err gate affords with ~6x margin:

- inputs: x^T chunks and the HOST-prescaled lhs (w*x)^T arrive as bf16,
  packed so each chunk is ONE DMA (10 chunk DMAs + 2 lhs DMAs instead of
  40+) — the HWDGE issue pipe (~625ns/DMA) would otherwise become the
  bottleneck once bytes halve.
- matmuls: bf16 runs 1 cyc/row at ANY output width (f32r needs >=256), so
  the e=0 / e=8 staircases trim all the way down to w=128.
- output: PSUM f32 is copied to bf16 and DMA'd as one [128,2048] transfer
  per full (stripe, e) block and one packed [128,1280] transfer per
  staircase; host upcasts while mirroring. Out bytes halve to 8.7 MB.
- diag: softplus chain (relu(z)+ln1p(exp(-|z|))) stays on device in f32 and
  ships as its own tiny [2,512] output; the host drops it onto the diagonal
  during assembly, which removes the eye/neye masks, the 8 SWDGE scatter
  DMAs and the diag->output dependency chain of v1.
- a few junk warmup matmuls run during the DMA lead-in so the PE p-state
  ramp (2x-slow first 3us of a busy stretch) is paid on junk, not work.

Per-core: PE ~58us busy (the N^2*D/2 half-Gram floor at 1 cyc/row), DMA
~15 MB = ~41us, DVE ~47us — PE-bound.
"""
import numpy as np
import concourse.bacc as bacc
import concourse.mybir as mybir
import concourse.tile as tile
from concourse.bass_utils import run_bass_kernel_spmd

N, D = 8192, 512
NCORES = 8
NST = 16                    # row stripes
SR = N // NST               # 512 rows per stripe
ECH = 9                     # chunks per stripe (cyclic distance 0..8)
MT = SR // 128              # 4 m-tiles per stripe
DB = D // 128               # 4 contraction blocks
NCHR = ECH + 1              # resident chunks 0..9

f32 = mybir.dt.float32
bf16 = mybir.dt.bfloat16
AF = mybir.ActivationFunctionType

# staircase packing: per trimmed block t, (packed col offset, width)
ST_OFF = [0, 512, 896, 1152]
ST_W = [512, 384, 256, 128]
ST_TOT = 1280
FULL_W = MT * 512           # 2048 cols per full (stripe, e) block
# per-stripe out cols: e=0 staircase | e=1..7 full | e=8 staircase
STRIPE_W = ST_TOT + 7 * FULL_W + ST_TOT     # 16896
OUT_W = 2 * STRIPE_W                        # 33792

NWARM_SMALL = 4             # early ap-128 junk matmuls (tiny memset ready first)
NWARM_BIG = 5               # ap-512 junk matmuls covering the PE p-state ramp

_cache: dict = {}


def _blk_off(si, e):
    """out128 column offset of block (stripe si, distance e)."""
    base = si * STRIPE_W
    if e == 0:
        return base
    if e == ECH - 1:
        return base + ST_TOT + 7 * FULL_W
    return base + ST_TOT + (e - 1) * FULL_W


def _build(reps=1):
    nc = bacc.Bacc("TRN2", target_bir_lowering=False, debug=False, num_devices=NCORES)
    # chunk-major packed x^T: chunk j at cols [2048j : 2048(j+1)], b-block b
    # at [2048j+512b : +512] (one DMA per chunk)
    xtc = nc.dram_tensor("xtc", [128, 512 * DB * NCHR], bf16, kind="ExternalInput")
    # host-prescaled lhsT = (w * x)^T, stripe si at cols [2048si : +2048]
    lhsd = nc.dram_tensor("lhsd", [128, 512 * DB * 2], bf16, kind="ExternalInput")
    vk2 = nc.dram_tensor("vk2", [128, DB], bf16, kind="ExternalInput")
    vb = nc.dram_tensor("vb", [128, 1], f32, kind="ExternalInput")
    out = nc.dram_tensor("out", [128, OUT_W], bf16, kind="ExternalOutput")
    dout = nc.dram_tensor("dout", [2, SR], f32, kind="ExternalOutput")

    with tile.TileContext(nc) as tc:
        with (
            tc.tile_pool(name="xt", bufs=1) as xt_pool,
            tc.tile_pool(name="lhs", bufs=1) as lhs_pool,
            tc.tile_pool(name="const", bufs=1) as cpool,
            tc.tile_pool(name="ps", bufs=6, space="PSUM") as ps_pool,
            tc.tile_pool(name="dps", bufs=2, space="PSUM") as dps_pool,
            tc.tile_pool(name="otf", bufs=6) as otf_pool,
            tc.tile_pool(name="ots", bufs=3) as ots_pool,
        ):
            xtall = xt_pool.tile([128, 512 * DB * NCHR], bf16, tag="xt", name="xt")
            lhst = lhs_pool.tile([128, 512 * DB * 2], bf16, tag="lhs", name="lhs")
            vkt = cpool.tile([128, DB], bf16, tag="vk")
            vbt = cpool.tile([128, 1], f32, tag="vb")
            wtile = cpool.tile([128, 640], bf16, tag="warm")
            vb1 = vbt[0:1, 0:1]

            def chunk(b, j):
                o = 2048 * j + 512 * b
                return xtall[:, o:o + 512]

            def lhsw(si, b, t):
                # t-major lhs packing: [si][t][b][128 cols] — lets stripe A's
                # t=0 weights arrive in a small early DMA
                o = 2048 * si + 512 * t + 128 * b
                return lhst[:, o:o + 128]

            def _emit():
                # tiny warmup scratch first on the Pool SWDGE queue, the big
                # half on DVE concurrently; then the consts
                nc.gpsimd.memset(wtile[:, 0:128], 0.125)
                nc.vector.memset(wtile[:, 128:640], 0.125)
                nc.gpsimd.dma_start(vkt[:], vk2[:])
                nc.gpsimd.dma_start(vbt[:], vb[:])
                # input stream, ordered so each PE phase's data lands just in
                # time: chunk0 in two halves (diag mm starts on b0/b1), then
                # stripe A's lhs per-t, chunk1, stripe B's lhs per-t, chunks 2..9
                def in_dma(o, w):
                    nc.sync.dma_start(xtall[:, o:o + w], xtc[:, o:o + w])

                def lhs_dma(o, w=512):
                    nc.sync.dma_start(lhst[:, o:o + w], lhsd[:, o:o + w])

                # interleaved so every PE phase's operands land just in time
                in_dma(0, 1024)          # chunk0 b0/b1
                in_dma(1024, 1024)       # chunk0 b2/b3
                lhs_dma(0)               # lhs A t0
                lhs_dma(512)             # lhs A t1
                in_dma(2048, 1024)       # chunk1 b0/b1
                lhs_dma(1024)            # lhs A t2
                lhs_dma(1536)            # lhs A t3
                in_dma(3072, 1024)       # chunk1 b2/b3
                for t in range(MT):
                    lhs_dma(2048 + 512 * t)
                for j in range(2, NCHR):
                    nc.sync.dma_start(
                        xtall[:, 2048 * j:2048 * (j + 1)],
                        xtc[:, 2048 * j:2048 * (j + 1)])

                # warm the PE p-state on junk during the DMA lead-in
                for _ in range(NWARM_SMALL):
                    pw = ps_pool.tile([128, 512], f32, tag="ps")
                    nc.tensor.matmul(pw[:, 0:128], wtile[:, 0:128], wtile[:, 0:128],
                                     start=True, stop=True)
                for _ in range(NWARM_BIG):
                    pw = ps_pool.tile([128, 512], f32, tag="ps")
                    nc.tensor.matmul(pw[:], wtile[:, 0:128], wtile[:, 128:640],
                                     start=True, stop=True)

                # diagonal rows: z[0, r] = x_stripe[r] . var_kernel, then
                # softplus(z+vb)+1e-8 = relu(z+vb) + ln(1+exp(-|z+vb|)) + 1e-8
                # (stripe A reads chunk 0, stripe B chunk 1)
                dq, dabs, drelu, drow = {}, {}, {}, {}
                for si in range(2):
                    dq[si] = dps_pool.tile([1, SR], f32, tag="dps", name=f"dps{si}")
                    for b in range(DB):
                        nc.tensor.matmul(dq[si][:], vkt[:, b:b + 1], chunk(b, si),
                                         start=(b == 0), stop=(b == DB - 1))
                    dabs[si] = cpool.tile([1, SR], f32, tag=f"dabs{si}", name=f"dabs{si}")
                    drelu[si] = cpool.tile([1, SR], f32, tag=f"drelu{si}", name=f"drelu{si}")
                    drow[si] = cpool.tile([1, SR], f32, tag=f"drow{si}", name=f"drow{si}")
                # one pass per ACT function (each LUT table-load happens once)
                for si in range(2):
                    nc.scalar.activation(dabs[si][:], dq[si][:], AF.Abs, bias=vb1)
                for si in range(2):
                    nc.scalar.activation(drelu[si][:], dq[si][:], AF.Relu, bias=vb1)
                for si in range(2):
                    nc.scalar.activation(dabs[si][:], dabs[si][:], AF.Exp, scale=-1.0)
                for si in range(2):
                    nc.scalar.activation(dabs[si][:], dabs[si][:], AF.Ln, bias=1.0)

                def stair_block(si, e, j, split_tail=False, act_copies=False):
                    """e=0 / e=8 staircase: only cols >= 128t of each m-tile,
                    packed tight into a [128, 1280] tile. split_tail issues
                    t0-t2 as soon as they're copied so the kernel's final
                    transfer is only the tiny [128,128] t3 piece. act_copies
                    alternates the PSUM->SBUF copies DVE/ACT so the tail
                    copies don't serialize on one engine."""
                    ot = ots_pool.tile([128, ST_TOT], bf16, tag="ots")
                    o = _blk_off(si, e)
                    for t in range(MT):
                        c0, w = 128 * t, ST_W[t]
                        p = ps_pool.tile([128, 512], f32, tag="ps")
                        for b in range(DB):
                            nc.tensor.matmul(
                                p[:, 0:w], lhsw(si, b, t),
                                chunk(b, j)[:, c0:512],
                                start=(b == 0), stop=(b == DB - 1))
                        dst = ot[:, ST_OFF[t]:ST_OFF[t] + w]
                        if act_copies and t % 2 == 1:
                            nc.scalar.activation(dst, p[:, 0:w], AF.Copy)
                        else:
                            nc.vector.tensor_copy(dst, p[:, 0:w])
                        if split_tail and t == 1:
                            # t0+t1 ship as soon as both are copied; t2+t3
                            # follow, so the final transfer is small and its
                            # issue isn't queued behind a big sibling
                            nc.sync.dma_start(
                                out[:, o:o + ST_OFF[2]], ot[:, 0:ST_OFF[2]])
                    if split_tail:
                        o2 = ST_OFF[2]
                        nc.sync.dma_start(out[:, o + o2:o + ST_TOT], ot[:, o2:ST_TOT])
                    else:
                        nc.sync.dma_start(out[:, o:o + ST_TOT], ot[:])

                def full_block(si, e, j, split_out=False):
                    ot = otf_pool.tile([128, FULL_W], bf16, tag="otf")
                    o = _blk_off(si, e)
                    for t in range(MT):
                        p = ps_pool.tile([128, 512], f32, tag="ps")
                        for b in range(DB):
                            nc.tensor.matmul(
                                p[:], lhsw(si, b, t),
                                chunk(b, j),
                                start=(b == 0), stop=(b == DB - 1))
                        nc.vector.tensor_copy(ot[:, 512 * t:512 * (t + 1)], p[:])
                        if split_out:
                            nc.sync.dma_start(
                                out[:, o + 512 * t:o + 512 * (t + 1)],
                                ot[:, 512 * t:512 * (t + 1)])
                    if not split_out:
                        nc.sync.dma_start(out[:, o:o + FULL_W], ot[:])

                # e=0 staircases first (their chunks land first)
                stair_block(0, 0, 0)
                stair_block(1, 0, 1)

                # finish the diag: drow = relu + ln1p + 1e-8, ship as [1,512].
                # Pool engine does the adds — DVE is saturated with copies and
                # these would head-of-line-block them behind the ACT chain.
                for si in range(2):
                    nc.gpsimd.tensor_add(drow[si][:], drelu[si][:], dabs[si][:])
                    nc.gpsimd.tensor_scalar_add(drow[si][:], drow[si][:], 1e-8)
                    nc.gpsimd.dma_start(dout[si:si + 1, :], drow[si][:])

                # main: chunk-major — chunk j unlocks stripe A's e=j and
                # stripe B's e=j-1 (B's chunk index for distance e is e+1)
                for j in range(1, NCHR):
                    items = ((0, j), (1, j - 1))
                    if j == ECH - 1:
                        # emit B:e7 (big DMA) before A:e8 so the final
                        # transfers are the small staircase pieces
                        items = ((1, j - 1), (0, j))
                    for si, e in items:
                        if e == 0 or e > ECH - 1:
                            continue
                        if e == ECH - 1:
                            stair_block(si, e, j, split_tail=(j == NCHR - 1),
                                        act_copies=True)
                        else:
                            full_block(si, e, j,
                                       split_out=(j >= ECH - 1))
            if reps == 1:
                _emit()
            else:
                ET = mybir.EngineType
                with tc.For_i(0, reps, 1, hint_engines=(
                        ET.PE, ET.SP, ET.DVE, ET.Activation, ET.Pool)):
                    _emit()
    nc.compile()
    return nc


def _get_nc():
    if "nc" not in _cache:
        _cache["nc"] = _build()
    return _cache["nc"]


def _make_in_maps(x, cov_kernel, var_kernel, var_bias):
    from ml_dtypes import bfloat16 as bf
    x = np.ascontiguousarray(x, dtype=np.float32)
    w = np.asarray(cov_kernel, np.float32)
    xT = np.ascontiguousarray(x.T)                       # [D, N]
    vk2 = np.ascontiguousarray(
        np.asarray(var_kernel, np.float32).reshape(DB, 128).T).astype(bf)
    vbt = np.full((128, 1), np.float32(np.asarray(var_bias).reshape(-1)[0]))
    in_maps = []
    for i in range(NCORES):
        off = 2 * i * SR
        xrot = np.concatenate([xT[:, off:], xT[:, :off]], axis=1) if off else xT
        xr = xrot[:, :512 * NCHR]                        # [512, 5120]
        xtc = np.ascontiguousarray(
            xr.reshape(DB, 128, NCHR, 512).transpose(1, 2, 0, 3)
            .reshape(128, 512 * DB * NCHR)).astype(bf)
        xws = w[:, None] * xrot[:, :2 * SR]              # [512, 1024] f32
        # t-major packing: lhsd[k, 2048si+512t+128b+m] = xws[128b+k, 512si+128t+m]
        lhsd = np.ascontiguousarray(
            xws.reshape(DB, 128, 2, MT, 128).transpose(1, 2, 3, 0, 4)
            .reshape(128, 512 * DB * 2)).astype(bf)
        in_maps.append({"xtc": xtc, "lhsd": lhsd, "vk2": vk2, "vb": vbt})
    return in_maps


def kernel(x, cov_kernel, var_kernel, var_bias):
    nc = _get_nc()
    in_maps = _make_in_maps(x, cov_kernel, var_kernel, var_bias)
    res = run_bass_kernel_spmd(nc, in_maps, core_ids=list(range(NCORES)))
    full = np.empty((N, N), dtype=np.float32)
    bands = []                                           # band[s] = [512, 4608]
    diags = []                                           # diag values per stripe
    for s in range(NST):
        arr = res.results[s // 2]["out"]
        si = s % 2
        band = np.empty((SR, ECH * 512), dtype=np.float32)
        for e in range(1, ECH - 1):
            o = _blk_off(si, e)
            band[:, 512 * e:512 * (e + 1)] = (
                arr[:, o:o + FULL_W].astype(np.float32)
                .reshape(128, MT, 512).swapaxes(0, 1).reshape(SR, 512))
        for e in (0, ECH - 1):
            o = _blk_off(si, e)
            for t in range(MT):
                c0, w = 128 * t, ST_W[t]
                band[c0:c0 + 128, 512 * e + c0:512 * e + c0 + w] = \
                    arr[:, o + ST_OFF[t]:o + ST_OFF[t] + w].astype(np.float32)
        bands.append(band)
        diags.append(res.results[s // 2]["dout"][si].astype(np.float32))
    for s in range(NST):
        b = bands[s]
        r0 = SR * s
        for e in range(ECH):
            u = (s + e) % NST
            full[r0:r0 + SR, SR * u:SR * (u + 1)] = b[:, SR * e:SR * (e + 1)]
        # diagonal block: device wrote only the upper staircase; mirror the
        # strictly-lower 128-sub-blocks from the transpose
        dblk = full[r0:r0 + SR, r0:r0 + SR]
        for t in range(1, MT):
            for u in range(t):
                dblk[128 * t:128 * (t + 1), 128 * u:128 * (u + 1)] = \
                    dblk[128 * u:128 * (u + 1), 128 * t:128 * (t + 1)].T
        # e=8 block (s, s+8): lower staircase comes from the partner
        # stripe's e=8 band (its sub-block (tb, ta), transposed)
        u8 = (s + ECH - 1) % NST
        eblk = full[r0:r0 + SR, SR * u8:SR * (u8 + 1)]
        bu = bands[u8]
        for t in range(1, MT):
            for u in range(t):
                eblk[128 * t:128 * (t + 1), 128 * u:128 * (u + 1)] = \
                    bu[128 * u:128 * (u + 1),
                       SR * (ECH - 1) + 128 * t:SR * (ECH - 1) + 128 * (t + 1)].T
        for e in range(ECH, NST):
            # mirror: block (s, u) = block (u, s)^T, distance 16-e from u
            u = (s + e) % NST
            full[r0:r0 + SR, SR * u:SR * (u + 1)] = \
                bands[u][:, SR * (NST - e):SR * (NST - e + 1)].T
    idx = np.arange(N)
    full[idx, idx] = np.concatenate(diags)
    return full
